# revision 10
# baseline (speedup 1.0000x reference)
"""Trainium2 Bass kernel for nn_DiffNet (gnn_message_passing).

The reference's per-element "edge MLP" over the meta stack
(vi, W, vj) -> two 1x1 convs -> weighted sum over the input dim is
linear in its 3 channels, so it collapses algebraically.  With
g = conv1_w.T @ conv2_w[0]  (3 scalars), hb = conv1_b@conv2_w[0]+conv2_b[0],
z = vi @ W.T (no bias), s1[b] = sum_i vi[b,i], s2[b] = sum_i vi[b,i]^2:

    out[b,o] = relu(z+b)[b,o] * (1 + scale*g2*s1[b])
             + scale*(g0*s2[b] + g1*z[b,o] + hb*s1[b])

so the whole network is 3 small matmuls + elementwise, and the problem
is memory-bound on the fc weights (3.5 MB fp32).

Distribution (8 cores, no collectives): fc1/fc2 replicated (any
zero-communication scheme must read them on every core since every
output depends on all of them), fc3 sharded over its output dim
(32 cols/core); full batch B=32 on every core; host concatenates the
8 [32,32] output shards.

On-core layout: activations live transposed [feature(partitions), batch]
in 128-row chunks; weights are passed pre-transposed [in, out] so matmuls
need no on-chip weight transpose.  Matmuls put the (tiny) activation
tile stationary and stream the weight chunk [128, 512] as the moving
operand in float32r (1 cycle/row at N>=512 vs 4 for plain fp32).  The
z output lands [batch, out]; a cheap PE transpose brings each 128-col
chunk back to [out, batch] where relu-bias (per-partition), the k1*z
term and the per-batch alpha/beta scalars (broadcast across partitions
via a rank-1 ones matmul) are applied with a handful of wide DVE ops.
"""

import sys

if "/opt/trn_rl_repo" not in sys.path:
    sys.path.insert(0, "/opt/trn_rl_repo")

import numpy as np


def _install_ntff_hook_shim():
    """This image's antenv lacks ``axon_hooks``; bass_utils hard-imports it
    when tracing under axon.  Provide the module and register the ctypes
    NTFF hook from trn_agent_boot so ``trace=True`` yields exec_time_ns."""
    import types

    if "antenv.axon_hooks" in sys.modules:
        return
    try:
        import antenv

        mod = types.ModuleType("antenv.axon_hooks")
        _h = [None]
        mod.set_axon_ntff_profile_hook = lambda hook: _h.__setitem__(0, hook)
        mod.get_axon_ntff_profile_hook = lambda: _h[0]
        sys.modules["antenv.axon_hooks"] = mod
        antenv.axon_hooks = mod
        from trn_agent_boot.trn_boot import _ntff_profile_via_ctypes

        mod.set_axon_ntff_profile_hook(
            _ntff_profile_via_ctypes("/opt/axon/libaxon_pjrt.so")
        )
    except Exception:
        pass


_install_ntff_hook_shim()

N_CORES = 8
B = 32
I1, O1, O2, O3 = 1024, 512, 512, 256
O3L = O3 // N_CORES  # fc3 output cols per core
RATE = 0.1

_CACHE = {}
LAST_RESULTS = None  # BassKernelResults of the most recent run (for test.py)


def _build(k0, k1, k2, kb):
    import concourse.bacc as bacc
    import concourse.mybir as mybir
    import concourse.tile as tile
    import concourse.bass as bass

    f32 = mybir.dt.float32
    # matmul operand dtype: plain fp32 for now (fp32r needs producer-side
    # rounding through the whole dataflow; revisit for PE throughput)
    f32r = mybir.dt.float32
    AF = mybir.ActivationFunctionType
    ALU = mybir.AluOpType

    nc = bacc.Bacc(
        "TRN2", target_bir_lowering=False, debug=False, num_devices=N_CORES
    )

    xt = nc.declare_dram_parameter("xt", [128, 8 * B], f32, isOutput=False)
    w1 = nc.declare_dram_parameter("w1t", [128, 8 * O1], f32, isOutput=False)
    w2 = nc.declare_dram_parameter("w2t", [128, 4 * O2], f32, isOutput=False)
    w3 = nc.declare_dram_parameter("w3t", [128, 4 * O3L], f32, isOutput=False)
    b12 = nc.declare_dram_parameter("b12", [128, 8], f32, isOutput=False)
    b3 = nc.declare_dram_parameter("b3", [O3L, 1], f32, isOutput=False)
    eye = nc.declare_dram_parameter("eye", [B, B], f32, isOutput=False)
    out_d = nc.declare_dram_parameter("out", [O3L, B], f32, isOutput=True)

    with tile.TileContext(nc) as tc:
        with (
            tc.tile_pool(name="wts", bufs=1) as wp,
            tc.tile_pool(name="act", bufs=1) as ap,
            tc.tile_pool(name="ps", bufs=2, space=bass.MemorySpace.PSUM) as pp,
        ):
            tw1 = wp.tile([128, 8 * O1], f32, tag="w1")
            tw2 = wp.tile([128, 4 * O2], f32, tag="w2")
            tw3 = wp.tile([128, 4 * O3L], f32, tag="w3")
            tb12 = wp.tile([128, 8], f32, tag="b12")
            tb3 = wp.tile([O3L, 1], f32, tag="b3")
            teye = wp.tile([B, B], f32, tag="eye")
            t1k = wp.tile([128, 1], f32, tag="onesk")  # K-dir ones (col sums)
            t1m = wp.tile([1, 128], f32, tag="onesm")  # bcast row -> 128 parts

            # x (+ its square) packed [p, (chunk, b)]; second half filled on-chip
            tx = ap.tile([128, 2 * 8 * B], f32, tag="a1")

            # -- DMAs.  Small/critical-path tensors on gpsimd (SWDGE);
            # weights on sync (HWDGE), w3 first so the last layer is
            # never gated on its (tiny) weights.
            nc.gpsimd.dma_start(tx[:, 0 : 8 * B], xt[:])
            nc.gpsimd.dma_start(tb12[:], b12[:])
            nc.gpsimd.dma_start(tb3[:], b3[:])
            nc.gpsimd.dma_start(teye[:], eye[:])
            nc.sync.dma_start(tw3[:], w3[:])
            for j in range(4):  # 512 KB each
                nc.sync.dma_start(
                    tw1[:, j * 2 * O1 : (j + 1) * 2 * O1],
                    w1[:, j * 2 * O1 : (j + 1) * 2 * O1],
                )
            for j in range(2):
                nc.sync.dma_start(
                    tw2[:, j * 2 * O2 : (j + 1) * 2 * O2],
                    w2[:, j * 2 * O2 : (j + 1) * 2 * O2],
                )

            nc.vector.memset(t1k[:], 1.0)
            nc.vector.memset(t1m[:], 1.0)

            def stats_ab(a_tile, n_c, tag):
                """a_tile [128, 2*n_c*B] = (a | a^2); -> ab_sb [128, 2*B]
                rows all equal; cols 0:B = alpha(b), B:2B = beta(b)."""
                # square the activation half
                nc.scalar.activation(
                    a_tile[:, n_c * B : 2 * n_c * B],
                    a_tile[:, 0 : n_c * B],
                    AF.Square,
                )
                s_ps = pp.tile([1, 2 * B], f32, tag="s")
                v4 = a_tile[:].rearrange("p (h c b) -> p h c b", h=2, c=n_c, b=B)
                for c in range(n_c):
                    nc.tensor.matmul(
                        s_ps[:].rearrange("p (h b) -> p h b", h=2),
                        t1k[:].bitcast(f32r),
                        v4[:, :, c, :].bitcast(f32r),
                        start=(c == 0),
                        stop=(c == n_c - 1),
                    )
                ab_row = ap.tile([1, 2 * B], f32, tag=tag + "row")
                tmp = ap.tile([1, B], f32, tag=tag + "tmp")
                # alpha = k2*s1 + 1
                nc.vector.tensor_scalar(
                    ab_row[:, 0:B], s_ps[:, 0:B], k2, 1.0, ALU.mult, ALU.add
                )
                # beta = k0*s2 + kb*s1
                nc.vector.tensor_scalar_mul(tmp[:], s_ps[:, B : 2 * B], k0)
                nc.vector.scalar_tensor_tensor(
                    ab_row[:, B : 2 * B],
                    s_ps[:, 0:B],
                    kb,
                    tmp[:],
                    ALU.mult,
                    ALU.add,
                )
                ab_ps = pp.tile([128, 2 * B], f32, tag="ab")
                nc.tensor.matmul(
                    ab_ps[:],
                    t1m[:].bitcast(f32r),
                    ab_row[:].bitcast(f32r),
                    start=True,
                    stop=True,
                )
                ab_sb = ap.tile([128, 2 * B], f32, tag=tag + "sb")
                nc.scalar.copy(ab_sb[:], ab_ps[:])
                return ab_sb

            def layer(a_tile, w_tile, n_ic, n_oc, ow, bias_col, ab_sb, out_view, li):
                """a_tile [128, >=n_ic*B] (activations in first n_ic*B cols),
                w_tile [128, n_ic*ow], ow = this layer's (local) output width,
                out_view [np_out, n_oc*B]."""
                np_out = min(ow, 128)  # partitions of transposed chunks
                z_ps = pp.tile([B, ow], f32, tag="z")
                for ic in range(n_ic):
                    nc.tensor.matmul(
                        z_ps[:],
                        a_tile[:, ic * B : (ic + 1) * B].bitcast(f32r),
                        w_tile[:, ic * ow : (ic + 1) * ow].bitcast(f32r),
                        start=(ic == 0),
                        stop=(ic == n_ic - 1),
                    )
                z_sb = ap.tile([B, ow], f32, tag=f"zsb{li}")
                nc.scalar.copy(z_sb[:], z_ps[:])
                zt_ps = pp.tile([np_out, n_oc * B], f32, tag="zt")
                vjt = ap.tile([np_out, n_oc * B], f32, tag=f"vj{li}")
                for oc in range(n_oc):
                    nc.tensor.transpose(
                        zt_ps[:, oc * B : (oc + 1) * B],
                        z_sb[:, oc * 128 : oc * 128 + np_out],
                        teye[:],
                    )
                    nc.scalar.activation(
                        vjt[:, oc * B : (oc + 1) * B],
                        zt_ps[:, oc * B : (oc + 1) * B],
                        AF.Relu,
                        bias=bias_col(oc),
                        scale=1.0,
                    )
                alpha = (
                    ab_sb[0:np_out, 0:B]
                    .rearrange("p (o b) -> p o b", o=1)
                    .broadcast_to((np_out, n_oc, B))
                )
                beta = (
                    ab_sb[0:np_out, B : 2 * B]
                    .rearrange("p (o b) -> p o b", o=1)
                    .broadcast_to((np_out, n_oc, B))
                )
                v3 = vjt[:].rearrange("p (o b) -> p o b", o=n_oc)
                z3 = zt_ps[:].rearrange("p (o b) -> p o b", o=n_oc)
                o3 = out_view.rearrange("p (o b) -> p o b", o=n_oc)
                t_sb = ap.tile([np_out, n_oc * B], f32, tag=f"t{li}")
                t3 = t_sb[:].rearrange("p (o b) -> p o b", o=n_oc)
                # t = k1*z + beta ; vj *= alpha ; out = vj*alpha + t
                nc.vector.scalar_tensor_tensor(
                    t3, z3, k1, beta, ALU.mult, ALU.add
                )
                nc.vector.tensor_tensor(v3, v3, alpha, ALU.mult)
                nc.vector.tensor_tensor(o3, v3, t3, ALU.add)

            # ---- forward chain
            ab1 = stats_ab(tx, 8, "ab1")
            a2 = ap.tile([128, 2 * 4 * B], f32, tag="a2")
            layer(tx, tw1, 8, 4, O1, lambda oc: tb12[:, oc : oc + 1], ab1,
                  a2[:, 0 : 4 * B], 1)

            ab2 = stats_ab(a2, 4, "ab2")
            a3 = ap.tile([128, 2 * 4 * B], f32, tag="a3")
            layer(a2, tw2, 4, 4, O2, lambda oc: tb12[:, 4 + oc : 5 + oc], ab2,
                  a3[:, 0 : 4 * B], 2)

            ab3 = stats_ab(a3, 4, "ab3")
            out_sb = ap.tile([O3L, B], f32, tag="o3")
            layer(a3, tw3, 4, 1, O3L, lambda oc: tb3[:, 0:1], ab3, out_sb[:], 3)

            nc.sync.dma_start(out_d[:], out_sb[:])

    nc.compile()
    return nc


def kernel(**inputs):
    from concourse.bass_utils import run_bass_kernel_spmd

    x = np.ascontiguousarray(np.asarray(inputs["x"], dtype=np.float32))
    fc1_w = np.asarray(inputs["fc1_w"], dtype=np.float32)
    fc1_b = np.asarray(inputs["fc1_b"], dtype=np.float32)
    fc2_w = np.asarray(inputs["fc2_w"], dtype=np.float32)
    fc2_b = np.asarray(inputs["fc2_b"], dtype=np.float32)
    fc3_w = np.asarray(inputs["fc3_w"], dtype=np.float32)
    fc3_b = np.asarray(inputs["fc3_b"], dtype=np.float32)
    c1w = np.asarray(inputs["conv1_w"], dtype=np.float32)
    c1b = np.asarray(inputs["conv1_b"], dtype=np.float32)
    c2w = np.asarray(inputs["conv2_w"], dtype=np.float32)
    c2b = np.asarray(inputs["conv2_b"], dtype=np.float32)
    bn = float(np.asarray(inputs["batch_num"]).astype(np.float64))

    scale = np.float32(RATE) / np.float32(bn)
    g = (c1w.T @ c2w[0]).astype(np.float32)  # [3]
    hb = np.float32(c1b @ c2w[0] + c2b[0])
    k0 = float(scale * g[0])
    k1 = float(scale * g[1])
    k2 = float(scale * g[2])
    kb = float(scale * hb)

    key = (k0, k1, k2, kb)
    if key not in _CACHE:
        _CACHE[key] = _build(*key)
    nc = _CACHE[key]

    def pack(m, n_c, width):  # [n_c*128, width] -> [128, n_c*width]
        return np.ascontiguousarray(
            m.reshape(n_c, 128, width).transpose(1, 0, 2).reshape(128, n_c * width)
        )

    xt_h = pack(x.T, 8, B)
    w1_h = pack(fc1_w.T, 8, O1)
    w2_h = pack(fc2_w.T, 4, O2)
    b12_h = np.ascontiguousarray(
        np.concatenate(
            [fc1_b.reshape(4, 128).T, fc2_b.reshape(4, 128).T], axis=1
        )
    )
    eye_h = np.eye(B, dtype=np.float32)

    in_maps = []
    for c in range(N_CORES):
        w3_h = pack(fc3_w[c * O3L : (c + 1) * O3L].T, 4, O3L)
        b3_h = np.ascontiguousarray(fc3_b[c * O3L : (c + 1) * O3L, None])
        in_maps.append(
            dict(
                xt=xt_h, w1t=w1_h, w2t=w2_h, w3t=w3_h,
                b12=b12_h, b3=b3_h, eye=eye_h,
            )
        )

    res = run_bass_kernel_spmd(nc, in_maps, list(range(N_CORES)))
    global LAST_RESULTS
    LAST_RESULTS = res
    return np.ascontiguousarray(
        np.concatenate([res.results[c]["out"].T for c in range(N_CORES)], axis=1)
    ).astype(np.float32)


if __name__ == "__main__":
    rng = np.random.default_rng(0)

    def lin(fo, fi):
        bound = 1.0 / np.sqrt(fi)
        return (
            rng.uniform(-bound, bound, (fo, fi)).astype(np.float32),
            rng.uniform(-bound, bound, (fo,)).astype(np.float32),
        )

    fc1_w, fc1_b = lin(512, 1024)
    fc2_w, fc2_b = lin(512, 512)
    fc3_w, fc3_b = lin(256, 512)
    c1w, c1b = lin(8, 3)
    c2w, c2b = lin(1, 8)
    ins = dict(
        x=rng.standard_normal((32, 1024)).astype(np.float32),
        fc1_w=fc1_w, fc1_b=fc1_b, fc2_w=fc2_w, fc2_b=fc2_b,
        fc3_w=fc3_w, fc3_b=fc3_b,
        conv1_w=c1w, conv1_b=c1b, conv2_w=c2w, conv2_b=c2b,
        batch_num=10,
    )
    out = kernel(**ins)
    print("kernel out", out.shape, out.dtype, float(np.abs(out).max()))


# revision 22
# speedup vs baseline: 1.0645x; 1.0645x over previous
"""Trainium2 Bass kernel for nn_DiffNet (gnn_message_passing).

The reference's per-element "edge MLP" over the meta stack
(vi, W, vj) -> two 1x1 convs -> weighted sum over the input dim is
linear in its 3 channels, so it collapses algebraically.  With
g = conv1_w.T @ conv2_w[0]  (3 scalars), hb = conv1_b@conv2_w[0]+conv2_b[0],
z = vi @ W.T (no bias), s1[b] = sum_i vi[b,i], s2[b] = sum_i vi[b,i]^2:

    out[b,o] = relu(z+b)[b,o] * (1 + scale*g2*s1[b])
             + scale*(g0*s2[b] + g1*z[b,o] + hb*s1[b])

so the whole network is 3 small matmuls + elementwise, and the problem
is memory-bound on the fc weights (3.5 MB fp32).

Distribution (8 cores, no collectives): fc1/fc2 replicated (any
zero-communication scheme must read them on every core since every
output depends on all of them), fc3 sharded over its output dim
(32 cols/core); full batch B=32 on every core; host concatenates the
8 [32,32] output shards.

On-core layout: activations live transposed [feature(partitions), batch]
in 128-row chunks; weights are passed pre-transposed [in, out] so matmuls
need no on-chip weight transpose.  Matmuls put the (tiny) activation
tile stationary and stream the weight chunk [128, 512] as the moving
operand in float32r (1 cycle/row at N>=512 vs 4 for plain fp32); all
tensors on the matmul dataflow are declared float32r so their producers
satisfy the walrus fp32r-rounding rule.  The z output lands
[batch, out]; a cheap PE transpose brings each 128-col chunk back to
[out, batch] where relu-bias (per-partition), the k1*z term and the
per-batch alpha/beta scalars (broadcast across partitions via a rank-1
ones matmul) are applied with a few wide DVE ops.
"""

import sys

if "/opt/trn_rl_repo" not in sys.path:
    sys.path.insert(0, "/opt/trn_rl_repo")

import numpy as np


def _install_ntff_hook_shim():
    """This image's antenv lacks ``axon_hooks``; bass_utils hard-imports it
    when tracing under axon.  Provide the module and register the ctypes
    NTFF hook from trn_agent_boot so ``trace=True`` yields exec_time_ns."""
    import types

    if "antenv.axon_hooks" in sys.modules:
        return
    try:
        import antenv

        mod = types.ModuleType("antenv.axon_hooks")
        _h = [None]
        mod.set_axon_ntff_profile_hook = lambda hook: _h.__setitem__(0, hook)
        mod.get_axon_ntff_profile_hook = lambda: _h[0]
        sys.modules["antenv.axon_hooks"] = mod
        antenv.axon_hooks = mod
        from trn_agent_boot.trn_boot import _ntff_profile_via_ctypes

        mod.set_axon_ntff_profile_hook(
            _ntff_profile_via_ctypes("/opt/axon/libaxon_pjrt.so")
        )
    except Exception:
        pass


_install_ntff_hook_shim()

N_CORES = 8
B = 32
I1, O1, O2, O3 = 1024, 512, 512, 256
O3L = O3 // N_CORES  # fc3 output cols per core
RATE = 0.1

_CACHE = {}
LAST_RESULTS = None  # BassKernelResults of the most recent run (for test.py)


def _build(k0, k1, k2, kb):
    import concourse.bacc as bacc
    import concourse.mybir as mybir
    import concourse.tile as tile
    import concourse.bass as bass

    f32 = mybir.dt.float32
    f32r = mybir.dt.float32r
    AF = mybir.ActivationFunctionType
    ALU = mybir.AluOpType

    nc = bacc.Bacc(
        "TRN2", target_bir_lowering=False, debug=False, num_devices=N_CORES
    )

    # misc layout (cols): [b12: 0..8) [b3: 8..9) [eye: 9..9+B)
    # [onesK col: 9+B] [onesM row: 10+B..10+B+128)
    MW = 10 + B + 128
    xt = nc.declare_dram_parameter("xt", [128, 8 * B], f32r, isOutput=False)
    misc = nc.declare_dram_parameter("misc", [128, MW], f32, isOutput=False)
    w1 = nc.declare_dram_parameter("w1t", [128, 8 * O1], f32r, isOutput=False)
    w2 = nc.declare_dram_parameter("w2t", [128, 4 * O2], f32r, isOutput=False)
    w3 = nc.declare_dram_parameter("w3t", [128, 4 * O3L], f32r, isOutput=False)
    out_d = nc.declare_dram_parameter("out", [O3L, B], f32, isOutput=True)

    with tile.TileContext(nc) as tc:
        with (
            tc.tile_pool(name="wts", bufs=1) as wp,
            tc.tile_pool(name="act", bufs=1) as ap,
            tc.tile_pool(name="ps", bufs=1, space=bass.MemorySpace.PSUM) as pp,
            tc.tile_pool(name="psz", bufs=2, space=bass.MemorySpace.PSUM) as ppz,
        ):
            tw1 = wp.tile([128, 8 * O1], f32r, tag="w1")
            tw2 = wp.tile([128, 4 * O2], f32r, tag="w2")
            tw3 = wp.tile([128, 4 * O3L], f32r, tag="w3")
            tmisc = wp.tile([128, MW], f32, tag="misc")
            tb12 = tmisc[:, 0:8]
            tb3 = tmisc[0:O3L, 8:9]
            teye = tmisc[0:B, 9 : 9 + B]
            t1k = tmisc[:, 9 + B : 10 + B]  # ones col (K-dir sums)
            t1m = tmisc[0:1, 10 + B : 10 + B + 128]  # ones row (bcast)

            # x packed [p, (chunk, b)]
            tx = ap.tile([128, 8 * B], f32r, tag="a1")

            # -- DMAs: two HWDGE rings (sync + scalar), small/critical first,
            # w3 early so the last layer is never gated on its weights.
            nc.sync.dma_start(tx[:], xt[:])
            for j in range(2):  # 1 MB halves of fc1
                nc.sync.dma_start(
                    tw1[:, j * 4 * O1 : (j + 1) * 4 * O1],
                    w1[:, j * 4 * O1 : (j + 1) * 4 * O1],
                )
            nc.scalar.dma_start(tmisc[:], misc[:])
            nc.scalar.dma_start(tw3[:], w3[:])
            nc.scalar.dma_start(tw2[:], w2[:])

            def stats_ab(a_tile, n_c, tag):
                """a_tile [128, n_c*B] float32r; -> ab_sb [128, 2*B]
                rows all equal; cols 0:B = alpha(b), B:2B = beta(b).
                All matmuls here are small-N: plain f32 (same PE cost).
                Squares live in a separate f32 tile (writes into a_tile
                must be fp32r per the walrus fp32r-producer rule)."""
                asq = ap.tile([128, n_c * B], f32, tag=tag + "sq")
                nc.scalar.activation(asq[:], a_tile[:].bitcast(f32), AF.Square)
                s1_ps = pp.tile([1, B], f32, tag="s1")
                s2_ps = pp.tile([1, B], f32, tag="s2")
                af = a_tile[:].bitcast(f32)
                for c in range(n_c):
                    nc.tensor.matmul(
                        s1_ps[:],
                        t1k,
                        af[:, c * B : (c + 1) * B],
                        start=(c == 0),
                        stop=(c == n_c - 1),
                    )
                for c in range(n_c):
                    nc.tensor.matmul(
                        s2_ps[:],
                        t1k,
                        asq[:, c * B : (c + 1) * B],
                        start=(c == 0),
                        stop=(c == n_c - 1),
                    )
                ab_row = ap.tile([1, 2 * B], f32, tag=tag + "row")
                tmp = ap.tile([1, B], f32, tag=tag + "tmp")
                # alpha = k2*s1 + 1
                nc.vector.tensor_scalar(
                    ab_row[:, 0:B], s1_ps[:], k2, 1.0, ALU.mult, ALU.add
                )
                # beta = k0*s2 + kb*s1
                nc.vector.tensor_scalar_mul(tmp[:], s2_ps[:], k0)
                nc.vector.scalar_tensor_tensor(
                    ab_row[:, B : 2 * B],
                    s1_ps[:],
                    kb,
                    tmp[:],
                    ALU.mult,
                    ALU.add,
                )
                ab_ps = pp.tile([128, 2 * B], f32, tag="ab")
                nc.tensor.matmul(
                    ab_ps[:], t1m, ab_row[:], start=True, stop=True
                )
                ab_sb = ap.tile([128, 2 * B], f32, tag=tag + "sb")
                nc.scalar.copy(ab_sb[:], ab_ps[:])
                return ab_sb

            def layer(a_tile, w_tile, n_ic, n_oc, ow, bias_col, ab_sb, out_view, li):
                """a_tile [128, >=n_ic*B] (activations in first n_ic*B cols),
                w_tile [128, n_ic*ow], ow = this layer's (local) output width,
                out_view [np_out, n_oc*B] (float32r tile slice)."""
                np_out = min(ow, 128)  # partitions of transposed chunks
                z_ps = ppz.tile([B, ow], f32, tag="z")
                for ic in range(n_ic):
                    nc.tensor.matmul(
                        z_ps[:],
                        a_tile[:, ic * B : (ic + 1) * B],
                        w_tile[:, ic * ow : (ic + 1) * ow],
                        start=(ic == 0),
                        stop=(ic == n_ic - 1),
                    )
                z_sb = ap.tile([B, ow], f32, tag=f"zsb{li}")
                nc.scalar.copy(z_sb[:], z_ps[:])
                zt_ps = pp.tile([np_out, n_oc * B], f32, tag="zt")
                vjt = ap.tile([np_out, n_oc * B], f32, tag=f"vj{li}")
                for oc in range(n_oc):
                    nc.tensor.transpose(
                        zt_ps[:, oc * B : (oc + 1) * B],
                        z_sb[:, oc * 128 : oc * 128 + np_out],
                        teye,
                    )
                    nc.scalar.activation(
                        vjt[:, oc * B : (oc + 1) * B],
                        zt_ps[:, oc * B : (oc + 1) * B],
                        AF.Relu,
                        bias=bias_col(oc),
                        scale=1.0,
                    )
                alpha = (
                    ab_sb[0:np_out, 0:B]
                    .rearrange("p (o b) -> p o b", o=1)
                    .broadcast_to((np_out, n_oc, B))
                )
                beta = (
                    ab_sb[0:np_out, B : 2 * B]
                    .rearrange("p (o b) -> p o b", o=1)
                    .broadcast_to((np_out, n_oc, B))
                )
                v3 = vjt[:].rearrange("p (o b) -> p o b", o=n_oc)
                z3 = zt_ps[:].rearrange("p (o b) -> p o b", o=n_oc)
                o3 = out_view.rearrange("p (o b) -> p o b", o=n_oc)
                t_sb = ap.tile([np_out, n_oc * B], f32, tag=f"t{li}")
                t3 = t_sb[:].rearrange("p (o b) -> p o b", o=n_oc)
                # t = k1*z + beta ; out = vj*alpha + t
                nc.vector.scalar_tensor_tensor(
                    t3, z3, k1, beta, ALU.mult, ALU.add
                )
                nc.vector.tensor_tensor(v3, v3, alpha, ALU.mult)
                nc.vector.tensor_tensor(o3, v3, t3, ALU.add)

            # ---- forward chain
            ab1 = stats_ab(tx, 8, "ab1")
            a2 = ap.tile([128, 4 * B], f32r, tag="a2")
            layer(tx, tw1, 8, 4, O1, lambda oc: tb12[:, oc : oc + 1], ab1,
                  a2[:], 1)

            ab2 = stats_ab(a2, 4, "ab2")
            a3 = ap.tile([128, 4 * B], f32r, tag="a3")
            layer(a2, tw2, 4, 4, O2, lambda oc: tb12[:, 4 + oc : 5 + oc], ab2,
                  a3[:], 2)

            ab3 = stats_ab(a3, 4, "ab3")
            out_sb = ap.tile([O3L, B], f32, tag="o3")
            layer(a3, tw3, 4, 1, O3L, lambda oc: tb3, ab3, out_sb[:], 3)

            nc.sync.dma_start(out_d[:], out_sb[:])

    nc.compile()
    return nc


def kernel(**inputs):
    from concourse.bass_utils import run_bass_kernel_spmd

    x = np.ascontiguousarray(np.asarray(inputs["x"], dtype=np.float32))
    fc1_w = np.asarray(inputs["fc1_w"], dtype=np.float32)
    fc1_b = np.asarray(inputs["fc1_b"], dtype=np.float32)
    fc2_w = np.asarray(inputs["fc2_w"], dtype=np.float32)
    fc2_b = np.asarray(inputs["fc2_b"], dtype=np.float32)
    fc3_w = np.asarray(inputs["fc3_w"], dtype=np.float32)
    fc3_b = np.asarray(inputs["fc3_b"], dtype=np.float32)
    c1w = np.asarray(inputs["conv1_w"], dtype=np.float32)
    c1b = np.asarray(inputs["conv1_b"], dtype=np.float32)
    c2w = np.asarray(inputs["conv2_w"], dtype=np.float32)
    c2b = np.asarray(inputs["conv2_b"], dtype=np.float32)
    bn = float(np.asarray(inputs["batch_num"]).astype(np.float64))

    scale = np.float32(RATE) / np.float32(bn)
    g = (c1w.T @ c2w[0]).astype(np.float32)  # [3]
    hb = np.float32(c1b @ c2w[0] + c2b[0])
    k0 = float(scale * g[0])
    k1 = float(scale * g[1])
    k2 = float(scale * g[2])
    kb = float(scale * hb)

    key = (k0, k1, k2, kb)
    if key not in _CACHE:
        _CACHE[key] = _build(*key)
    nc = _CACHE[key]

    def pack(m, n_c, width):  # [n_c*128, width] -> [128, n_c*width]
        return np.ascontiguousarray(
            m.reshape(n_c, 128, width).transpose(1, 0, 2).reshape(128, n_c * width)
        )

    xt_h = pack(x.T, 8, B)
    w1_h = pack(fc1_w.T, 8, O1)
    w2_h = pack(fc2_w.T, 4, O2)
    MW = 10 + B + 128
    misc_h = np.zeros((128, MW), dtype=np.float32)
    misc_h[:, 0:4] = fc1_b.reshape(4, 128).T
    misc_h[:, 4:8] = fc2_b.reshape(4, 128).T
    misc_h[0:B, 9 : 9 + B] = np.eye(B, dtype=np.float32)
    misc_h[:, 9 + B] = 1.0  # ones col (K-dir sums)
    misc_h[:, 10 + B : 10 + B + 128] = 1.0  # ones row (partition bcast)

    in_maps = []
    for c in range(N_CORES):
        w3_h = pack(fc3_w[c * O3L : (c + 1) * O3L].T, 4, O3L)
        m_h = misc_h.copy()
        m_h[0:O3L, 8] = fc3_b[c * O3L : (c + 1) * O3L]
        in_maps.append(
            dict(xt=xt_h, w1t=w1_h, w2t=w2_h, w3t=w3_h, misc=m_h)
        )

    res = run_bass_kernel_spmd(nc, in_maps, list(range(N_CORES)))
    global LAST_RESULTS
    LAST_RESULTS = res
    return np.ascontiguousarray(
        np.concatenate([res.results[c]["out"].T for c in range(N_CORES)], axis=1)
    ).astype(np.float32)


if __name__ == "__main__":
    rng = np.random.default_rng(0)

    def lin(fo, fi):
        bound = 1.0 / np.sqrt(fi)
        return (
            rng.uniform(-bound, bound, (fo, fi)).astype(np.float32),
            rng.uniform(-bound, bound, (fo,)).astype(np.float32),
        )

    fc1_w, fc1_b = lin(512, 1024)
    fc2_w, fc2_b = lin(512, 512)
    fc3_w, fc3_b = lin(256, 512)
    c1w, c1b = lin(8, 3)
    c2w, c2b = lin(1, 8)
    ins = dict(
        x=rng.standard_normal((32, 1024)).astype(np.float32),
        fc1_w=fc1_w, fc1_b=fc1_b, fc2_w=fc2_w, fc2_b=fc2_b,
        fc3_w=fc3_w, fc3_b=fc3_b,
        conv1_w=c1w, conv1_b=c1b, conv2_w=c2w, conv2_b=c2b,
        batch_num=10,
    )
    out = kernel(**ins)
    print("kernel out", out.shape, out.dtype, float(np.abs(out).max()))


# revision 24
# speedup vs baseline: 1.0796x; 1.0142x over previous
"""Trainium2 Bass kernel for nn_DiffNet (gnn_message_passing).

The reference's per-element "edge MLP" over the meta stack
(vi, W, vj) -> two 1x1 convs -> weighted sum over the input dim is
linear in its 3 channels, so it collapses algebraically.  With
g = conv1_w.T @ conv2_w[0]  (3 scalars), hb = conv1_b@conv2_w[0]+conv2_b[0],
z = vi @ W.T (no bias), s1[b] = sum_i vi[b,i], s2[b] = sum_i vi[b,i]^2:

    out[b,o] = relu(z+b)[b,o] * (1 + scale*g2*s1[b])
             + scale*(g0*s2[b] + g1*z[b,o] + hb*s1[b])

so the whole network is 3 small matmuls + elementwise, and the problem
is memory-bound on the fc weights (3.5 MB fp32).

Distribution (8 cores, no collectives): fc1/fc2 replicated (any
zero-communication scheme must read them on every core since every
output depends on all of them), fc3 sharded over its output dim
(32 cols/core); full batch B=32 on every core; host concatenates the
8 [32,32] output shards.

On-core layout: activations live transposed [feature(partitions), batch]
in 128-row chunks; weights are passed pre-transposed [in, out] so matmuls
need no on-chip weight transpose.  Matmuls put the (tiny) activation
tile stationary and stream the weight chunk [128, 512] as the moving
operand in float32r (1 cycle/row at N>=512 vs 4 for plain fp32); all
tensors on the matmul dataflow are declared float32r so their producers
satisfy the walrus fp32r-rounding rule.  The z output lands
[batch, out]; a cheap PE transpose brings each 128-col chunk back to
[out, batch] where relu-bias (per-partition), the k1*z term and the
per-batch alpha/beta scalars (broadcast across partitions via a rank-1
ones matmul) are applied with a few wide DVE ops.
"""

import sys

if "/opt/trn_rl_repo" not in sys.path:
    sys.path.insert(0, "/opt/trn_rl_repo")

import numpy as np


def _install_ntff_hook_shim():
    """This image's antenv lacks ``axon_hooks``; bass_utils hard-imports it
    when tracing under axon.  Provide the module and register the ctypes
    NTFF hook from trn_agent_boot so ``trace=True`` yields exec_time_ns."""
    import types

    if "antenv.axon_hooks" in sys.modules:
        return
    try:
        import antenv

        mod = types.ModuleType("antenv.axon_hooks")
        _h = [None]
        mod.set_axon_ntff_profile_hook = lambda hook: _h.__setitem__(0, hook)
        mod.get_axon_ntff_profile_hook = lambda: _h[0]
        sys.modules["antenv.axon_hooks"] = mod
        antenv.axon_hooks = mod
        from trn_agent_boot.trn_boot import _ntff_profile_via_ctypes

        mod.set_axon_ntff_profile_hook(
            _ntff_profile_via_ctypes("/opt/axon/libaxon_pjrt.so")
        )
    except Exception:
        pass


_install_ntff_hook_shim()

N_CORES = 8
B = 32
I1, O1, O2, O3 = 1024, 512, 512, 256
O3L = O3 // N_CORES  # fc3 output cols per core
RATE = 0.1

_CACHE = {}
LAST_RESULTS = None  # BassKernelResults of the most recent run (for test.py)


def _build(k0, k1, k2, kb):
    import concourse.bacc as bacc
    import concourse.mybir as mybir
    import concourse.tile as tile
    import concourse.bass as bass

    f32 = mybir.dt.float32
    f32r = mybir.dt.float32r
    AF = mybir.ActivationFunctionType
    ALU = mybir.AluOpType

    from concourse.tile_rust import add_dep_helper

    nc = bacc.Bacc(
        "TRN2", target_bir_lowering=False, debug=False, num_devices=N_CORES
    )

    # misc layout (cols): [b12: 0..8) [b3: 8..9) [eye: 9..9+B)
    # [onesK col: 9+B] [onesM row: 10+B..10+B+128)
    MW = 10 + B + 128
    xt = nc.declare_dram_parameter("xt", [128, 8 * B], f32r, isOutput=False)
    misc = nc.declare_dram_parameter("misc", [128, MW], f32, isOutput=False)
    w1 = nc.declare_dram_parameter("w1t", [128, 8 * O1], f32r, isOutput=False)
    w2 = nc.declare_dram_parameter("w2t", [128, 4 * O2], f32r, isOutput=False)
    w3 = nc.declare_dram_parameter("w3t", [128, 4 * O3L], f32r, isOutput=False)
    out_d = nc.declare_dram_parameter("out", [O3L, B], f32, isOutput=True)

    with tile.TileContext(nc) as tc:
        with (
            tc.tile_pool(name="wts", bufs=1) as wp,
            tc.tile_pool(name="act", bufs=1) as ap,
            tc.tile_pool(name="ps", bufs=1, space=bass.MemorySpace.PSUM) as pp,
            tc.tile_pool(name="psz", bufs=2, space=bass.MemorySpace.PSUM) as ppz,
        ):
            tw1 = wp.tile([128, 8 * O1], f32r, tag="w1")
            tw2 = wp.tile([128, 4 * O2], f32r, tag="w2")
            tw3 = wp.tile([128, 4 * O3L], f32r, tag="w3")
            tmisc = wp.tile([128, MW], f32, tag="misc")
            tb12 = tmisc[:, 0:8]
            tb3 = tmisc[0:O3L, 8:9]
            teye = tmisc[0:B, 9 : 9 + B]
            t1k = tmisc[:, 9 + B : 10 + B]  # ones col (K-dir sums)
            t1m = tmisc[0:1, 10 + B : 10 + B + 128]  # ones row (bcast)

            # x packed [p, (chunk, b)]
            tx = ap.tile([128, 8 * B], f32r, tag="a1")

            # -- DMAs: two HWDGE rings (sync + scalar), small/critical first,
            # w3 early so the last layer is never gated on its weights.
            # Weights arrive in chunks so each z-matmul gates only on its
            # own slice of the stream.
            nc.sync.dma_start(tmisc[:], misc[:])
            nc.sync.dma_start(tx[:], xt[:])
            for j in range(4):  # 512 KB quarters of fc1
                nc.sync.dma_start(
                    tw1[:, j * 2 * O1 : (j + 1) * 2 * O1],
                    w1[:, j * 2 * O1 : (j + 1) * 2 * O1],
                )
            nc.scalar.dma_start(tw3[:], w3[:])
            for j in range(2):
                nc.scalar.dma_start(
                    tw2[:, j * 2 * O2 : (j + 1) * 2 * O2],
                    w2[:, j * 2 * O2 : (j + 1) * 2 * O2],
                )

            def ordered(dependent, dependency, why):
                if dependent is not None and dependency is not None:
                    add_dep_helper(
                        dependent.ins, dependency.ins, sync=False, reason=why
                    )

            def stats_ab(a_tile, n_c, tag, after_mm=None):
                """a_tile [128, n_c*B] float32r; -> (ab_sb [128, 2*B], bcast).
                ab rows all equal; cols 0:B = alpha(b), B:2B = beta(b).
                All matmuls here are small-N: plain f32 (same PE cost).
                Squares live in a separate f32 tile (writes into a_tile
                must be fp32r per the walrus fp32r-producer rule)."""
                asq = ap.tile([128, n_c * B], f32, tag=tag + "sq")
                nc.scalar.activation(asq[:], a_tile[:].bitcast(f32), AF.Square)
                s1_ps = pp.tile([1, B], f32, tag="s1")
                s2_ps = pp.tile([1, B], f32, tag="s2")
                af = a_tile[:].bitcast(f32)
                first = None
                for c in range(n_c):
                    mm = nc.tensor.matmul(
                        s1_ps[:],
                        t1k,
                        af[:, c * B : (c + 1) * B],
                        start=(c == 0),
                        stop=(c == n_c - 1),
                    )
                    first = first or mm
                for c in range(n_c):
                    nc.tensor.matmul(
                        s2_ps[:],
                        t1k,
                        asq[:, c * B : (c + 1) * B],
                        start=(c == 0),
                        stop=(c == n_c - 1),
                    )
                ordered(first, after_mm, "stats after this layer's z matmuls")
                ab_row = ap.tile([1, 2 * B], f32, tag=tag + "row")
                tmp = ap.tile([1, B], f32, tag=tag + "tmp")
                # alpha = k2*s1 + 1
                nc.vector.tensor_scalar(
                    ab_row[:, 0:B], s1_ps[:], k2, 1.0, ALU.mult, ALU.add
                )
                # beta = k0*s2 + kb*s1
                nc.vector.tensor_scalar_mul(tmp[:], s2_ps[:], k0)
                nc.vector.scalar_tensor_tensor(
                    ab_row[:, B : 2 * B],
                    s1_ps[:],
                    kb,
                    tmp[:],
                    ALU.mult,
                    ALU.add,
                )
                ab_ps = pp.tile([128, 2 * B], f32, tag="ab")
                bcast = nc.tensor.matmul(
                    ab_ps[:], t1m, ab_row[:], start=True, stop=True
                )
                ab_sb = ap.tile([128, 2 * B], f32, tag=tag + "sb")
                nc.scalar.copy(ab_sb[:], ab_ps[:])
                return ab_sb, bcast

            def z_mms(a_tile, w_tile, n_ic, ow, after=None):
                """z_ps [B, ow] = a.T @ w, accumulated over n_ic chunks."""
                z_ps = ppz.tile([B, ow], f32, tag="z")
                last = None
                for ic in range(n_ic):
                    mm = nc.tensor.matmul(
                        z_ps[:],
                        a_tile[:, ic * B : (ic + 1) * B],
                        w_tile[:, ic * ow : (ic + 1) * ow],
                        start=(ic == 0),
                        stop=(ic == n_ic - 1),
                    )
                    if ic == 0:
                        ordered(mm, after, "z matmuls after stats bcast")
                    last = mm
                return z_ps, last

            def tail(z_ps, n_oc, ow, bias_col, ab_sb, out_view, li, after=None):
                """transpose z back to [out, batch]; relu+bias on DVE;
                combine with alpha/beta; writes out_view [np_out, n_oc*B]."""
                np_out = min(ow, 128)
                z_sb = ap.tile([B, ow], f32, tag=f"zsb{li}")
                zt_ps = pp.tile([np_out, n_oc * B], f32, tag="zt")
                vjt = ap.tile([np_out, n_oc * B], f32, tag=f"vj{li}")
                for oc in range(n_oc):
                    sl = slice(oc * 128, oc * 128 + np_out)
                    nc.scalar.copy(z_sb[:, sl], z_ps[:, sl])
                    tr = nc.tensor.transpose(
                        zt_ps[:, oc * B : (oc + 1) * B], z_sb[:, sl], teye
                    )
                    if oc == 0:
                        ordered(tr, after, "transposes after stats bcast")
                    # relu(z + bias) on DVE: per-partition bias, then max 0
                    nc.vector.tensor_scalar(
                        vjt[:, oc * B : (oc + 1) * B],
                        zt_ps[:, oc * B : (oc + 1) * B],
                        bias_col(oc),
                        0.0,
                        ALU.add,
                        ALU.max,
                    )
                alpha = (
                    ab_sb[0:np_out, 0:B]
                    .rearrange("p (o b) -> p o b", o=1)
                    .broadcast_to((np_out, n_oc, B))
                )
                beta = (
                    ab_sb[0:np_out, B : 2 * B]
                    .rearrange("p (o b) -> p o b", o=1)
                    .broadcast_to((np_out, n_oc, B))
                )
                v3 = vjt[:].rearrange("p (o b) -> p o b", o=n_oc)
                zt3 = zt_ps[:].rearrange("p (o b) -> p o b", o=n_oc)
                o3 = out_view.rearrange("p (o b) -> p o b", o=n_oc)
                t_sb = ap.tile([np_out, n_oc * B], f32, tag=f"t{li}")
                t3 = t_sb[:].rearrange("p (o b) -> p o b", o=n_oc)
                # t = k1*z + beta ; out = vj*alpha + t
                nc.vector.scalar_tensor_tensor(
                    t3, zt3, k1, beta, ALU.mult, ALU.add
                )
                nc.vector.tensor_tensor(v3, v3, alpha, ALU.mult)
                nc.vector.tensor_tensor(o3, v3, t3, ALU.add)

            # ---- forward chain: stats1 fills the PE while fc1 streams in;
            # later layers run stats between their z matmuls and transposes.
            ab1, bc1 = stats_ab(tx, 8, "ab1")
            z1, z1l = z_mms(tx, tw1, 8, O1, after=bc1)
            a2 = ap.tile([128, 4 * B], f32r, tag="a2")
            tail(z1, 4, O1, lambda oc: tb12[:, oc : oc + 1], ab1, a2[:], 1)

            z2, z2l = z_mms(a2, tw2, 4, O2)
            ab2, bc2 = stats_ab(a2, 4, "ab2", after_mm=z2l)
            a3 = ap.tile([128, 4 * B], f32r, tag="a3")
            tail(z2, 4, O2, lambda oc: tb12[:, 4 + oc : 5 + oc], ab2, a3[:], 2,
                 after=bc2)

            z3, z3l = z_mms(a3, tw3, 4, O3L)
            ab3, bc3 = stats_ab(a3, 4, "ab3", after_mm=z3l)
            out_sb = ap.tile([O3L, B], f32, tag="o3")
            tail(z3, 1, O3L, lambda oc: tb3, ab3, out_sb[:], 3, after=bc3)

            nc.sync.dma_start(out_d[:], out_sb[:])

    nc.compile()
    return nc


def kernel(**inputs):
    from concourse.bass_utils import run_bass_kernel_spmd

    x = np.ascontiguousarray(np.asarray(inputs["x"], dtype=np.float32))
    fc1_w = np.asarray(inputs["fc1_w"], dtype=np.float32)
    fc1_b = np.asarray(inputs["fc1_b"], dtype=np.float32)
    fc2_w = np.asarray(inputs["fc2_w"], dtype=np.float32)
    fc2_b = np.asarray(inputs["fc2_b"], dtype=np.float32)
    fc3_w = np.asarray(inputs["fc3_w"], dtype=np.float32)
    fc3_b = np.asarray(inputs["fc3_b"], dtype=np.float32)
    c1w = np.asarray(inputs["conv1_w"], dtype=np.float32)
    c1b = np.asarray(inputs["conv1_b"], dtype=np.float32)
    c2w = np.asarray(inputs["conv2_w"], dtype=np.float32)
    c2b = np.asarray(inputs["conv2_b"], dtype=np.float32)
    bn = float(np.asarray(inputs["batch_num"]).astype(np.float64))

    scale = np.float32(RATE) / np.float32(bn)
    g = (c1w.T @ c2w[0]).astype(np.float32)  # [3]
    hb = np.float32(c1b @ c2w[0] + c2b[0])
    k0 = float(scale * g[0])
    k1 = float(scale * g[1])
    k2 = float(scale * g[2])
    kb = float(scale * hb)

    key = (k0, k1, k2, kb)
    if key not in _CACHE:
        _CACHE[key] = _build(*key)
    nc = _CACHE[key]

    def pack(m, n_c, width):  # [n_c*128, width] -> [128, n_c*width]
        return np.ascontiguousarray(
            m.reshape(n_c, 128, width).transpose(1, 0, 2).reshape(128, n_c * width)
        )

    xt_h = pack(x.T, 8, B)
    w1_h = pack(fc1_w.T, 8, O1)
    w2_h = pack(fc2_w.T, 4, O2)
    MW = 10 + B + 128
    misc_h = np.zeros((128, MW), dtype=np.float32)
    misc_h[:, 0:4] = fc1_b.reshape(4, 128).T
    misc_h[:, 4:8] = fc2_b.reshape(4, 128).T
    misc_h[0:B, 9 : 9 + B] = np.eye(B, dtype=np.float32)
    misc_h[:, 9 + B] = 1.0  # ones col (K-dir sums)
    misc_h[:, 10 + B : 10 + B + 128] = 1.0  # ones row (partition bcast)

    in_maps = []
    for c in range(N_CORES):
        w3_h = pack(fc3_w[c * O3L : (c + 1) * O3L].T, 4, O3L)
        m_h = misc_h.copy()
        m_h[0:O3L, 8] = fc3_b[c * O3L : (c + 1) * O3L]
        in_maps.append(
            dict(xt=xt_h, w1t=w1_h, w2t=w2_h, w3t=w3_h, misc=m_h)
        )

    res = run_bass_kernel_spmd(nc, in_maps, list(range(N_CORES)))
    global LAST_RESULTS
    LAST_RESULTS = res
    return np.ascontiguousarray(
        np.concatenate([res.results[c]["out"].T for c in range(N_CORES)], axis=1)
    ).astype(np.float32)


if __name__ == "__main__":
    rng = np.random.default_rng(0)

    def lin(fo, fi):
        bound = 1.0 / np.sqrt(fi)
        return (
            rng.uniform(-bound, bound, (fo, fi)).astype(np.float32),
            rng.uniform(-bound, bound, (fo,)).astype(np.float32),
        )

    fc1_w, fc1_b = lin(512, 1024)
    fc2_w, fc2_b = lin(512, 512)
    fc3_w, fc3_b = lin(256, 512)
    c1w, c1b = lin(8, 3)
    c2w, c2b = lin(1, 8)
    ins = dict(
        x=rng.standard_normal((32, 1024)).astype(np.float32),
        fc1_w=fc1_w, fc1_b=fc1_b, fc2_w=fc2_w, fc2_b=fc2_b,
        fc3_w=fc3_w, fc3_b=fc3_b,
        conv1_w=c1w, conv1_b=c1b, conv2_w=c2w, conv2_b=c2b,
        batch_num=10,
    )
    out = kernel(**ins)
    print("kernel out", out.shape, out.dtype, float(np.abs(out).max()))


# revision 40
# speedup vs baseline: 1.2616x; 1.1686x over previous
"""Trainium2 Bass kernel for nn_DiffNet (gnn_message_passing).

The reference's per-element "edge MLP" over the meta stack
(vi, W, vj) -> two 1x1 convs -> weighted sum over the input dim is
linear in its 3 channels, so it collapses algebraically.  With
g = conv1_w.T @ conv2_w[0]  (3 scalars), hb = conv1_b@conv2_w[0]+conv2_b[0],
z = vi @ W.T (no bias), s1[b] = sum_i vi[b,i], s2[b] = sum_i vi[b,i]^2:

    out[b,o] = relu(z+b)[b,o] * (1 + scale*g2*s1[b])
             + scale*(g0*s2[b] + g1*z[b,o] + hb*s1[b])

so the whole network is 3 small matmuls + elementwise, and the problem
is memory-bound on the fc weights (3.5 MB fp32).

Distribution (8 cores, no collectives): fc1/fc2 replicated (any
zero-communication scheme must read them on every core since every
output depends on all of them), fc3 sharded over its output dim
(32 cols/core); full batch B=32 on every core; host concatenates the
8 [32,32] output shards.

On-core layout: activations live transposed [feature(partitions), batch]
in 128-row chunks; weights are passed pre-transposed [in, out] so matmuls
need no on-chip weight transpose.  Matmuls put the (tiny) activation
tile stationary and stream the weight chunk [128, 512] as the moving
operand in float32r (1 cycle/row at N>=512 vs 4 for plain fp32); all
tensors on the matmul dataflow are declared float32r so their producers
satisfy the walrus fp32r-rounding rule.  The z output lands
[batch, out]; a cheap PE transpose brings each 128-col chunk back to
[out, batch] where relu-bias (per-partition), the k1*z term and the
per-batch alpha/beta scalars (broadcast across partitions via a rank-1
ones matmul) are applied with a few wide DVE ops.
"""

import sys

if "/opt/trn_rl_repo" not in sys.path:
    sys.path.insert(0, "/opt/trn_rl_repo")

import numpy as np


def _install_ntff_hook_shim():
    """This image's antenv lacks ``axon_hooks``; bass_utils hard-imports it
    when tracing under axon.  Provide the module and register the ctypes
    NTFF hook from trn_agent_boot so ``trace=True`` yields exec_time_ns."""
    import types

    if "antenv.axon_hooks" in sys.modules:
        return
    try:
        import antenv

        mod = types.ModuleType("antenv.axon_hooks")
        _h = [None]
        mod.set_axon_ntff_profile_hook = lambda hook: _h.__setitem__(0, hook)
        mod.get_axon_ntff_profile_hook = lambda: _h[0]
        sys.modules["antenv.axon_hooks"] = mod
        antenv.axon_hooks = mod
        from trn_agent_boot.trn_boot import _ntff_profile_via_ctypes

        mod.set_axon_ntff_profile_hook(
            _ntff_profile_via_ctypes("/opt/axon/libaxon_pjrt.so")
        )
    except Exception:
        pass


_install_ntff_hook_shim()

N_CORES = 8
B = 32
I1, O1, O2, O3 = 1024, 512, 512, 256
O3L = O3 // N_CORES  # fc3 output cols per core
RATE = 0.1

_CACHE = {}
LAST_RESULTS = None  # BassKernelResults of the most recent run (for test.py)


def _build(k0, k1, k2, kb):
    import concourse.bacc as bacc
    import concourse.mybir as mybir
    import concourse.tile as tile
    import concourse.bass as bass

    f32 = mybir.dt.float32
    f32r = mybir.dt.float32r
    AF = mybir.ActivationFunctionType
    ALU = mybir.AluOpType

    from concourse.tile_rust import add_dep_helper

    nc = bacc.Bacc(
        "TRN2", target_bir_lowering=False, debug=False, num_devices=N_CORES
    )

    f16 = mybir.dt.float16
    # x is f32r (the DMA *rounds* f32r payloads — only matmul operands may
    # travel that way); everything else rides a plain-f32 misc tensor.
    # misc cols: [b12: 0..8) [b3: 8] [eye: 9..41) [onesK: 41]
    # [onesM fp16 row packed in f32: 42..106)
    XW = 8 * B
    MW = 42 + 64
    xm = nc.declare_dram_parameter("xm", [128, XW], f32r, isOutput=False)
    misc = nc.declare_dram_parameter("misc", [128, MW], f32, isOutput=False)
    w1 = nc.declare_dram_parameter("w1t", [128, 8 * O1], f32r, isOutput=False)
    w2 = nc.declare_dram_parameter("w2t", [128, 4 * O2], f32r, isOutput=False)
    w3 = nc.declare_dram_parameter("w3t", [128, 4 * O3L], f32r, isOutput=False)
    out_d = nc.declare_dram_parameter("out", [O3L, B], f32, isOutput=True)

    with tile.TileContext(nc) as tc:
        with (
            tc.tile_pool(name="wts", bufs=1) as wp,
            tc.tile_pool(name="act", bufs=1) as ap,
            tc.tile_pool(name="ps", bufs=1, space=bass.MemorySpace.PSUM) as pp,
        ):
            tw1 = wp.tile([128, 8 * O1], f32r, tag="w1")
            tw2 = wp.tile([128, 4 * O2], f32r, tag="w2")
            tw3 = wp.tile([128, 4 * O3L], f32r, tag="w3")
            txm = wp.tile([128, XW], f32r, tag="xm")
            tx = txm[:]  # f32r activations for layer 1
            tmisc = wp.tile([128, MW], f32, tag="misc")
            tb12 = tmisc[:, 0:8]
            tb3 = tmisc[0:O3L, 8:9]
            teye = tmisc[0:B, 9:41]
            t1k = tmisc[:, 41:42]  # f32 ones col (K-dir sums)
            t1m16 = tmisc[0:1, 42:106].bitcast(f16)  # [1,128] f16 ones

            # -- DMAs: one HWDGE ring, in need-order, few enough that each
            # gets its own completion-sem lane.  fc1 in thirds so its
            # z-matmuls start as the stream lands.
            nc.sync.dma_start(tmisc[:], misc[:])
            nc.sync.dma_start(txm[:], xm[:])
            for lo, hi in ((0, 3), (3, 6), (6, 8)):
                nc.sync.dma_start(
                    tw1[:, lo * O1 : hi * O1], w1[:, lo * O1 : hi * O1]
                )
            nc.sync.dma_start(tw2[:], w2[:])
            nc.sync.dma_start(tw3[:], w3[:])

            def ordered(dependent, dependency, why):
                if dependent is not None and dependency is not None:
                    add_dep_helper(
                        dependent.ins, dependency.ins, sync=False, reason=why
                    )

            def stats_ab(a_tile, n_c, tag, after_mm=None):
                """a_tile [128, n_c*B] float32r; -> (ab_sb [128, 2*B], bcast).
                ab rows all equal; cols 0:B = alpha(b), B:2B = beta(b).
                Everything on the f32r single-pass path: squares come from a
                DVE multiply writing f32r (the walrus fp32r-producer rule
                allows DVE outputs), so both column-sum chains are f32r."""
                asq = ap.tile([128, n_c * B], f32r, tag=tag + "sq")
                af = a_tile.bitcast(f32)
                nc.vector.tensor_tensor(asq[:], af, af, ALU.mult)
                s1_ps = pp.tile([1, B], f32, tag="s1")
                s2_ps = pp.tile([1, B], f32, tag="s2")
                mm1 = None
                for c in range(n_c):
                    mm = nc.tensor.matmul(
                        s1_ps[:],
                        t1k,
                        af[:, c * B : (c + 1) * B],
                        start=(c == 0),
                        stop=(c == n_c - 1),
                    )
                    mm1 = mm1 or mm
                asqf = asq[:].bitcast(f32)
                for c in range(n_c):
                    nc.tensor.matmul(
                        s2_ps[:],
                        t1k,
                        asqf[:, c * B : (c + 1) * B],
                        start=(c == 0),
                        stop=(c == n_c - 1),
                    )
                ordered(mm1, after_mm, "stats after this layer's z matmuls")
                ab_row = ap.tile([1, 2 * B], f16, tag=tag + "row")
                tmp = ap.tile([1, B], f32, tag=tag + "tmp")
                # alpha = k2*s1 + 1
                nc.vector.tensor_scalar(
                    ab_row[:, 0:B], s1_ps[:], k2, 1.0, ALU.mult, ALU.add
                )
                # beta = k0*s2 + kb*s1
                nc.vector.tensor_scalar_mul(tmp[:], s2_ps[:], k0)
                nc.vector.scalar_tensor_tensor(
                    ab_row[:, B : 2 * B],
                    s1_ps[:],
                    kb,
                    tmp[:],
                    ALU.mult,
                    ALU.add,
                )
                ab_ps = pp.tile([128, 2 * B], f32, tag="ab")
                bcast = nc.tensor.matmul(
                    ab_ps[:], t1m16, ab_row[:], start=True, stop=True
                )
                ab_sb = ap.tile([128, 2 * B], f32, tag=tag + "sb")
                nc.scalar.copy(ab_sb[:], ab_ps[:])
                return ab_sb, bcast

            def z_mms(a_tile, w_tile, n_ic, ow, after=None):
                """z_ps [B, ow] = a.T @ w, accumulated over n_ic chunks."""
                z_ps = pp.tile([B, ow], f32, tag="z")
                last = None
                for ic in range(n_ic):
                    mm = nc.tensor.matmul(
                        z_ps[:],
                        a_tile[:, ic * B : (ic + 1) * B],
                        w_tile[:, ic * ow : (ic + 1) * ow],
                        start=(ic == 0),
                        stop=(ic == n_ic - 1),
                    )
                    if ic == 0:
                        ordered(mm, after, "z matmuls after stats bcast")
                    last = mm
                return z_ps, last

            def tail(z_ps, n_oc, ow, bias_col, ab_sb, out_view, li, after=None):
                """transpose z back to [out, batch]; relu+bias on DVE;
                combine with alpha/beta; writes out_view [np_out, n_oc*B]."""
                np_out = min(ow, 128)
                z_sb = ap.tile([B, ow], f32, tag=f"zsb{li}")
                # one PSUM bank per oc so PE transpose-writes and DVE
                # relu-reads of different oc chunks don't serialize on
                # Tile's bank-overlap tracking
                zt_ps = pp.tile([np_out, n_oc, 512], f32, tag="zt")
                vjt = ap.tile([np_out, n_oc * B], f32, tag=f"vj{li}")
                for oc in range(n_oc):
                    sl = slice(oc * 128, oc * 128 + np_out)
                    nc.scalar.copy(z_sb[:, sl], z_ps[:, sl])
                    tr = nc.tensor.transpose(
                        zt_ps[:, oc, 0:B], z_sb[:, sl], teye
                    )
                    if oc == 0:
                        ordered(tr, after, "transposes after stats bcast")
                    # relu(z + bias) on DVE: per-partition bias, then max 0
                    nc.vector.tensor_scalar(
                        vjt[:, oc * B : (oc + 1) * B],
                        zt_ps[:, oc, 0:B],
                        bias_col(oc),
                        0.0,
                        ALU.add,
                        ALU.max,
                    )
                alpha = (
                    ab_sb[0:np_out, 0:B]
                    .rearrange("p (o b) -> p o b", o=1)
                    .broadcast_to((np_out, n_oc, B))
                )
                beta = (
                    ab_sb[0:np_out, B : 2 * B]
                    .rearrange("p (o b) -> p o b", o=1)
                    .broadcast_to((np_out, n_oc, B))
                )
                v3 = vjt[:].rearrange("p (o b) -> p o b", o=n_oc)
                zt3 = zt_ps[:, :, 0:B]
                o3 = out_view.rearrange("p (o b) -> p o b", o=n_oc)
                t_sb = ap.tile([np_out, n_oc * B], f32, tag=f"t{li}")
                t3 = t_sb[:].rearrange("p (o b) -> p o b", o=n_oc)
                # t = k1*z + beta ; out = vj*alpha + t
                nc.vector.scalar_tensor_tensor(
                    t3, zt3, k1, beta, ALU.mult, ALU.add
                )
                nc.vector.tensor_tensor(v3, v3, alpha, ALU.mult)
                nc.vector.tensor_tensor(o3, v3, t3, ALU.add)

            # ---- forward chain: stats1 fills the PE while fc1 streams in;
            # later layers run stats between their z matmuls and transposes.
            ab1, bc1 = stats_ab(tx, 8, "ab1")
            z1, z1l = z_mms(tx, tw1, 8, O1, after=bc1)
            a2 = ap.tile([128, 4 * B], f32r, tag="a2")
            tail(z1, 4, O1, lambda oc: tb12[:, oc : oc + 1], ab1, a2[:], 1)

            z2, z2l = z_mms(a2[:], tw2, 4, O2)
            ab2, bc2 = stats_ab(a2[:], 4, "ab2", after_mm=z2l)
            a3 = ap.tile([128, 4 * B], f32r, tag="a3")
            tail(z2, 4, O2, lambda oc: tb12[:, 4 + oc : 5 + oc], ab2, a3[:], 2,
                 after=bc2)

            z3, z3l = z_mms(a3[:], tw3, 4, O3L)
            ab3, bc3 = stats_ab(a3[:], 4, "ab3", after_mm=z3l)
            out_sb = ap.tile([O3L, B], f32, tag="o3")
            tail(z3, 1, O3L, lambda oc: tb3, ab3, out_sb[:], 3, after=bc3)

            nc.sync.dma_start(out_d[:], out_sb[:])

    nc.compile()
    return nc


def kernel(**inputs):
    from concourse.bass_utils import run_bass_kernel_spmd

    x = np.ascontiguousarray(np.asarray(inputs["x"], dtype=np.float32))
    fc1_w = np.asarray(inputs["fc1_w"], dtype=np.float32)
    fc1_b = np.asarray(inputs["fc1_b"], dtype=np.float32)
    fc2_w = np.asarray(inputs["fc2_w"], dtype=np.float32)
    fc2_b = np.asarray(inputs["fc2_b"], dtype=np.float32)
    fc3_w = np.asarray(inputs["fc3_w"], dtype=np.float32)
    fc3_b = np.asarray(inputs["fc3_b"], dtype=np.float32)
    c1w = np.asarray(inputs["conv1_w"], dtype=np.float32)
    c1b = np.asarray(inputs["conv1_b"], dtype=np.float32)
    c2w = np.asarray(inputs["conv2_w"], dtype=np.float32)
    c2b = np.asarray(inputs["conv2_b"], dtype=np.float32)
    bn = float(np.asarray(inputs["batch_num"]).astype(np.float64))

    scale = np.float32(RATE) / np.float32(bn)
    g = (c1w.T @ c2w[0]).astype(np.float32)  # [3]
    hb = np.float32(c1b @ c2w[0] + c2b[0])
    k0 = float(scale * g[0])
    k1 = float(scale * g[1])
    k2 = float(scale * g[2])
    kb = float(scale * hb)

    key = (k0, k1, k2, kb)
    if key not in _CACHE:
        _CACHE[key] = _build(*key)
    nc = _CACHE[key]

    def pack(m, n_c, width):  # [n_c*128, width] -> [128, n_c*width]
        return np.ascontiguousarray(
            m.reshape(n_c, 128, width).transpose(1, 0, 2).reshape(128, n_c * width)
        )

    w1_h = pack(fc1_w.T, 8, O1)
    w2_h = pack(fc2_w.T, 4, O2)
    xm_h = pack(x.T, 8, B)
    # misc layout must match _build: b12 | b3 | eye | onesK | onesM(f16)
    MW = 42 + 64
    misc_h = np.zeros((128, MW), dtype=np.float32)
    misc_h[:, 0:4] = fc1_b.reshape(4, 128).T
    misc_h[:, 4:8] = fc2_b.reshape(4, 128).T
    misc_h[0:B, 9:41] = np.eye(B, dtype=np.float32)
    misc_h[:, 41] = 1.0  # ones col (K-dir sums)
    misc_h[:, 42:106] = np.ones(128, np.float16).view(np.float32)[None, :]

    in_maps = []
    for c in range(N_CORES):
        w3_h = pack(fc3_w[c * O3L : (c + 1) * O3L].T, 4, O3L)
        m_h = misc_h.copy()
        m_h[0:O3L, 8] = fc3_b[c * O3L : (c + 1) * O3L]
        in_maps.append(
            dict(xm=xm_h, misc=m_h, w1t=w1_h, w2t=w2_h, w3t=w3_h)
        )

    res = run_bass_kernel_spmd(nc, in_maps, list(range(N_CORES)))
    global LAST_RESULTS
    LAST_RESULTS = res
    return np.ascontiguousarray(
        np.concatenate([res.results[c]["out"].T for c in range(N_CORES)], axis=1)
    ).astype(np.float32)


if __name__ == "__main__":
    rng = np.random.default_rng(0)

    def lin(fo, fi):
        bound = 1.0 / np.sqrt(fi)
        return (
            rng.uniform(-bound, bound, (fo, fi)).astype(np.float32),
            rng.uniform(-bound, bound, (fo,)).astype(np.float32),
        )

    fc1_w, fc1_b = lin(512, 1024)
    fc2_w, fc2_b = lin(512, 512)
    fc3_w, fc3_b = lin(256, 512)
    c1w, c1b = lin(8, 3)
    c2w, c2b = lin(1, 8)
    ins = dict(
        x=rng.standard_normal((32, 1024)).astype(np.float32),
        fc1_w=fc1_w, fc1_b=fc1_b, fc2_w=fc2_w, fc2_b=fc2_b,
        fc3_w=fc3_w, fc3_b=fc3_b,
        conv1_w=c1w, conv1_b=c1b, conv2_w=c2w, conv2_b=c2b,
        batch_num=10,
    )
    out = kernel(**ins)
    print("kernel out", out.shape, out.dtype, float(np.abs(out).max()))


# revision 41
# speedup vs baseline: 1.2688x; 1.0057x over previous
"""Trainium2 Bass kernel for nn_DiffNet (gnn_message_passing).

The reference's per-element "edge MLP" over the meta stack
(vi, W, vj) -> two 1x1 convs -> weighted sum over the input dim is
linear in its 3 channels, so it collapses algebraically.  With
g = conv1_w.T @ conv2_w[0]  (3 scalars), hb = conv1_b@conv2_w[0]+conv2_b[0],
z = vi @ W.T (no bias), s1[b] = sum_i vi[b,i], s2[b] = sum_i vi[b,i]^2:

    out[b,o] = relu(z+b)[b,o] * (1 + scale*g2*s1[b])
             + scale*(g0*s2[b] + g1*z[b,o] + hb*s1[b])

so the whole network is 3 small matmuls + elementwise, and the problem
is memory-bound on the fc weights (3.5 MB fp32).

Distribution (8 cores, no collectives): fc1/fc2 replicated (any
zero-communication scheme must read them on every core since every
output depends on all of them), fc3 sharded over its output dim
(32 cols/core); full batch B=32 on every core; host concatenates the
8 [32,32] output shards.

On-core layout: activations live transposed [feature(partitions), batch]
in 128-row chunks; weights are passed pre-transposed [in, out] so matmuls
need no on-chip weight transpose.  Matmuls put the (tiny) activation
tile stationary and stream the weight chunk [128, 512] as the moving
operand in float32r (1 cycle/row at N>=512 vs 4 for plain fp32); all
tensors on the matmul dataflow are declared float32r so their producers
satisfy the walrus fp32r-rounding rule.  The z output lands
[batch, out]; a cheap PE transpose brings each 128-col chunk back to
[out, batch] where relu-bias (per-partition), the k1*z term and the
per-batch alpha/beta scalars (broadcast across partitions via a rank-1
ones matmul) are applied with a few wide DVE ops.
"""

import sys

if "/opt/trn_rl_repo" not in sys.path:
    sys.path.insert(0, "/opt/trn_rl_repo")

import numpy as np


def _install_ntff_hook_shim():
    """This image's antenv lacks ``axon_hooks``; bass_utils hard-imports it
    when tracing under axon.  Provide the module and register the ctypes
    NTFF hook from trn_agent_boot so ``trace=True`` yields exec_time_ns."""
    import types

    if "antenv.axon_hooks" in sys.modules:
        return
    try:
        import antenv

        mod = types.ModuleType("antenv.axon_hooks")
        _h = [None]
        mod.set_axon_ntff_profile_hook = lambda hook: _h.__setitem__(0, hook)
        mod.get_axon_ntff_profile_hook = lambda: _h[0]
        sys.modules["antenv.axon_hooks"] = mod
        antenv.axon_hooks = mod
        from trn_agent_boot.trn_boot import _ntff_profile_via_ctypes

        mod.set_axon_ntff_profile_hook(
            _ntff_profile_via_ctypes("/opt/axon/libaxon_pjrt.so")
        )
    except Exception:
        pass


_install_ntff_hook_shim()

N_CORES = 8
B = 32
I1, O1, O2, O3 = 1024, 512, 512, 256
O3L = O3 // N_CORES  # fc3 output cols per core
RATE = 0.1

_CACHE = {}
LAST_RESULTS = None  # BassKernelResults of the most recent run (for test.py)


def _build(k0, k1, k2, kb):
    import concourse.bacc as bacc
    import concourse.mybir as mybir
    import concourse.tile as tile
    import concourse.bass as bass

    f32 = mybir.dt.float32
    f32r = mybir.dt.float32r
    AF = mybir.ActivationFunctionType
    ALU = mybir.AluOpType

    from concourse.tile_rust import add_dep_helper

    nc = bacc.Bacc(
        "TRN2", target_bir_lowering=False, debug=False, num_devices=N_CORES
    )

    f16 = mybir.dt.float16
    # x is f32r (the DMA *rounds* f32r payloads — only matmul operands may
    # travel that way); everything else rides a plain-f32 misc tensor.
    # misc cols: [b12: 0..8) [b3: 8] [eye: 9..41) [onesK: 41]
    # [onesM fp16 row packed in f32: 42..106)
    XW = 8 * B
    MW = 42 + 64
    xm = nc.declare_dram_parameter("xm", [128, XW], f32r, isOutput=False)
    misc = nc.declare_dram_parameter("misc", [128, MW], f32, isOutput=False)
    w1 = nc.declare_dram_parameter("w1t", [128, 8 * O1], f32r, isOutput=False)
    w2 = nc.declare_dram_parameter("w2t", [128, 4 * O2], f32r, isOutput=False)
    w3 = nc.declare_dram_parameter("w3t", [128, 4 * O3L], f32r, isOutput=False)
    out_d = nc.declare_dram_parameter("out", [O3L, B], f32, isOutput=True)

    with tile.TileContext(nc) as tc:
        with (
            tc.tile_pool(name="wts", bufs=1) as wp,
            tc.tile_pool(name="act", bufs=1) as ap,
            tc.tile_pool(name="ps", bufs=1, space=bass.MemorySpace.PSUM) as pp,
        ):
            tw1 = wp.tile([128, 8 * O1], f32r, tag="w1")
            tw2 = wp.tile([128, 4 * O2], f32r, tag="w2")
            tw3 = wp.tile([128, 4 * O3L], f32r, tag="w3")
            txm = wp.tile([128, XW], f32r, tag="xm")
            tx = txm[:]  # f32r activations for layer 1
            tmisc = wp.tile([128, MW], f32, tag="misc")
            tb12 = tmisc[:, 0:8]
            tb3 = tmisc[0:O3L, 8:9]
            teye = tmisc[0:B, 9:41]
            t1k = tmisc[:, 41:42]  # f32 ones col (K-dir sums)
            t1m16 = tmisc[0:1, 42:106].bitcast(f16)  # [1,128] f16 ones

            # -- DMAs: one HWDGE ring, in need-order, few enough that each
            # gets its own completion-sem lane.  fc1 in thirds so its
            # z-matmuls start as the stream lands.
            nc.sync.dma_start(tmisc[:], misc[:])
            nc.sync.dma_start(txm[:], xm[:])
            for lo, hi in ((0, 3), (3, 6), (6, 8)):
                nc.sync.dma_start(
                    tw1[:, lo * O1 : hi * O1], w1[:, lo * O1 : hi * O1]
                )
            nc.sync.dma_start(tw2[:], w2[:])
            nc.sync.dma_start(tw3[:], w3[:])

            def ordered(dependent, dependency, why):
                if dependent is not None and dependency is not None:
                    add_dep_helper(
                        dependent.ins, dependency.ins, sync=False, reason=why
                    )

            def stats_ab(a_tile, n_c, tag, after_mm=None):
                """a_tile [128, n_c*B] float32r; -> (ab_sb [128, 2*B], bcast).
                ab rows all equal; cols 0:B = alpha(b), B:2B = beta(b).
                Everything on the f32r single-pass path: squares come from a
                DVE multiply writing f32r (the walrus fp32r-producer rule
                allows DVE outputs), so both column-sum chains are f32r."""
                asq = ap.tile([128, n_c * B], f32r, tag=tag + "sq")
                af = a_tile.bitcast(f32)
                nc.vector.tensor_tensor(asq[:], af, af, ALU.mult)
                s1_ps = pp.tile([1, B], f32, tag="s1")
                s2_ps = pp.tile([1, B], f32, tag="s2")
                mm1 = None
                for c in range(n_c):
                    mm = nc.tensor.matmul(
                        s1_ps[:],
                        t1k,
                        af[:, c * B : (c + 1) * B],
                        start=(c == 0),
                        stop=(c == n_c - 1),
                    )
                    mm1 = mm1 or mm
                asqf = asq[:].bitcast(f32)
                for c in range(n_c):
                    nc.tensor.matmul(
                        s2_ps[:],
                        t1k,
                        asqf[:, c * B : (c + 1) * B],
                        start=(c == 0),
                        stop=(c == n_c - 1),
                    )
                ordered(mm1, after_mm, "stats after this layer's z matmuls")
                ab_row = ap.tile([1, 2 * B], f16, tag=tag + "row")
                tmp = ap.tile([1, B], f32, tag=tag + "tmp")
                # alpha = k2*s1 + 1
                nc.vector.tensor_scalar(
                    ab_row[:, 0:B], s1_ps[:], k2, 1.0, ALU.mult, ALU.add
                )
                # beta = k0*s2 + kb*s1
                nc.vector.tensor_scalar_mul(tmp[:], s2_ps[:], k0)
                nc.vector.scalar_tensor_tensor(
                    ab_row[:, B : 2 * B],
                    s1_ps[:],
                    kb,
                    tmp[:],
                    ALU.mult,
                    ALU.add,
                )
                ab_ps = pp.tile([128, 2 * B], f32, tag="ab")
                bcast = nc.tensor.matmul(
                    ab_ps[:], t1m16, ab_row[:], start=True, stop=True
                )
                ab_sb = ap.tile([128, 2 * B], f32, tag=tag + "sb")
                nc.scalar.copy(ab_sb[:], ab_ps[:])
                return ab_sb, bcast

            def z_mms(a_tile, w_tile, n_ic, ow, after=None):
                """z_ps [B, ow] = a.T @ w, accumulated over n_ic chunks."""
                z_ps = pp.tile([B, ow], f32, tag="z")
                last = None
                for ic in range(n_ic):
                    mm = nc.tensor.matmul(
                        z_ps[:],
                        a_tile[:, ic * B : (ic + 1) * B],
                        w_tile[:, ic * ow : (ic + 1) * ow],
                        start=(ic == 0),
                        stop=(ic == n_ic - 1),
                    )
                    if ic == 0:
                        ordered(mm, after, "z matmuls after stats bcast")
                    last = mm
                return z_ps, last

            def tail(z_ps, n_oc, ow, bias_col, ab_sb, out_view, li, after=None):
                """transpose z back to [out, batch]; relu+bias on DVE;
                combine with alpha/beta; writes out_view [np_out, n_oc*B]."""
                np_out = min(ow, 128)
                z_sb = ap.tile([B, ow], f32, tag=f"zsb{li}")
                nc.scalar.copy(z_sb[:], z_ps[:])
                # one PSUM bank per oc so PE transpose-writes and DVE
                # relu-reads of different oc chunks don't serialize on
                # Tile's bank-overlap tracking
                zt_ps = pp.tile([np_out, n_oc, 512], f32, tag="zt")
                vjt = ap.tile([np_out, n_oc * B], f32, tag=f"vj{li}")
                t_sb = ap.tile([np_out, n_oc * B], f32, tag=f"t{li}")
                alpha = ab_sb[0:np_out, 0:B]
                beta = ab_sb[0:np_out, B : 2 * B]
                for oc in range(n_oc):
                    bsl = slice(oc * B, (oc + 1) * B)
                    tr = nc.tensor.transpose(
                        zt_ps[:, oc, 0:B],
                        z_sb[:, oc * 128 : oc * 128 + np_out],
                        teye,
                    )
                    if oc == 0:
                        ordered(tr, after, "transposes after stats bcast")
                    # relu(z + bias) on DVE: per-partition bias, then max 0
                    nc.vector.tensor_scalar(
                        vjt[:, bsl],
                        zt_ps[:, oc, 0:B],
                        bias_col(oc),
                        0.0,
                        ALU.add,
                        ALU.max,
                    )
                    # t = k1*z + beta ; out = vj*alpha + t  (per-oc so the
                    # next layer's matmul ic can start as soon as its input
                    # chunk exists)
                    nc.vector.scalar_tensor_tensor(
                        t_sb[:, bsl], zt_ps[:, oc, 0:B], k1, beta,
                        ALU.mult, ALU.add,
                    )
                    nc.vector.tensor_tensor(
                        vjt[:, bsl], vjt[:, bsl], alpha, ALU.mult
                    )
                    nc.vector.tensor_tensor(
                        out_view[:, bsl], vjt[:, bsl], t_sb[:, bsl], ALU.add
                    )

            # ---- forward chain: stats1 fills the PE while fc1 streams in;
            # later layers run stats between their z matmuls and transposes.
            ab1, bc1 = stats_ab(tx, 8, "ab1")
            z1, z1l = z_mms(tx, tw1, 8, O1, after=bc1)
            a2 = ap.tile([128, 4 * B], f32r, tag="a2")
            tail(z1, 4, O1, lambda oc: tb12[:, oc : oc + 1], ab1, a2[:], 1)

            z2, z2l = z_mms(a2[:], tw2, 4, O2)
            ab2, bc2 = stats_ab(a2[:], 4, "ab2", after_mm=z2l)
            a3 = ap.tile([128, 4 * B], f32r, tag="a3")
            tail(z2, 4, O2, lambda oc: tb12[:, 4 + oc : 5 + oc], ab2, a3[:], 2,
                 after=bc2)

            z3, z3l = z_mms(a3[:], tw3, 4, O3L)
            ab3, bc3 = stats_ab(a3[:], 4, "ab3", after_mm=z3l)
            out_sb = ap.tile([O3L, B], f32, tag="o3")
            tail(z3, 1, O3L, lambda oc: tb3, ab3, out_sb[:], 3, after=bc3)

            nc.sync.dma_start(out_d[:], out_sb[:])

    nc.compile()
    return nc


def kernel(**inputs):
    from concourse.bass_utils import run_bass_kernel_spmd

    x = np.ascontiguousarray(np.asarray(inputs["x"], dtype=np.float32))
    fc1_w = np.asarray(inputs["fc1_w"], dtype=np.float32)
    fc1_b = np.asarray(inputs["fc1_b"], dtype=np.float32)
    fc2_w = np.asarray(inputs["fc2_w"], dtype=np.float32)
    fc2_b = np.asarray(inputs["fc2_b"], dtype=np.float32)
    fc3_w = np.asarray(inputs["fc3_w"], dtype=np.float32)
    fc3_b = np.asarray(inputs["fc3_b"], dtype=np.float32)
    c1w = np.asarray(inputs["conv1_w"], dtype=np.float32)
    c1b = np.asarray(inputs["conv1_b"], dtype=np.float32)
    c2w = np.asarray(inputs["conv2_w"], dtype=np.float32)
    c2b = np.asarray(inputs["conv2_b"], dtype=np.float32)
    bn = float(np.asarray(inputs["batch_num"]).astype(np.float64))

    scale = np.float32(RATE) / np.float32(bn)
    g = (c1w.T @ c2w[0]).astype(np.float32)  # [3]
    hb = np.float32(c1b @ c2w[0] + c2b[0])
    k0 = float(scale * g[0])
    k1 = float(scale * g[1])
    k2 = float(scale * g[2])
    kb = float(scale * hb)

    key = (k0, k1, k2, kb)
    if key not in _CACHE:
        _CACHE[key] = _build(*key)
    nc = _CACHE[key]

    def pack(m, n_c, width):  # [n_c*128, width] -> [128, n_c*width]
        return np.ascontiguousarray(
            m.reshape(n_c, 128, width).transpose(1, 0, 2).reshape(128, n_c * width)
        )

    w1_h = pack(fc1_w.T, 8, O1)
    w2_h = pack(fc2_w.T, 4, O2)
    xm_h = pack(x.T, 8, B)
    # misc layout must match _build: b12 | b3 | eye | onesK | onesM(f16)
    MW = 42 + 64
    misc_h = np.zeros((128, MW), dtype=np.float32)
    misc_h[:, 0:4] = fc1_b.reshape(4, 128).T
    misc_h[:, 4:8] = fc2_b.reshape(4, 128).T
    misc_h[0:B, 9:41] = np.eye(B, dtype=np.float32)
    misc_h[:, 41] = 1.0  # ones col (K-dir sums)
    misc_h[:, 42:106] = np.ones(128, np.float16).view(np.float32)[None, :]

    in_maps = []
    for c in range(N_CORES):
        w3_h = pack(fc3_w[c * O3L : (c + 1) * O3L].T, 4, O3L)
        m_h = misc_h.copy()
        m_h[0:O3L, 8] = fc3_b[c * O3L : (c + 1) * O3L]
        in_maps.append(
            dict(xm=xm_h, misc=m_h, w1t=w1_h, w2t=w2_h, w3t=w3_h)
        )

    res = run_bass_kernel_spmd(nc, in_maps, list(range(N_CORES)))
    global LAST_RESULTS
    LAST_RESULTS = res
    return np.ascontiguousarray(
        np.concatenate([res.results[c]["out"].T for c in range(N_CORES)], axis=1)
    ).astype(np.float32)


if __name__ == "__main__":
    rng = np.random.default_rng(0)

    def lin(fo, fi):
        bound = 1.0 / np.sqrt(fi)
        return (
            rng.uniform(-bound, bound, (fo, fi)).astype(np.float32),
            rng.uniform(-bound, bound, (fo,)).astype(np.float32),
        )

    fc1_w, fc1_b = lin(512, 1024)
    fc2_w, fc2_b = lin(512, 512)
    fc3_w, fc3_b = lin(256, 512)
    c1w, c1b = lin(8, 3)
    c2w, c2b = lin(1, 8)
    ins = dict(
        x=rng.standard_normal((32, 1024)).astype(np.float32),
        fc1_w=fc1_w, fc1_b=fc1_b, fc2_w=fc2_w, fc2_b=fc2_b,
        fc3_w=fc3_w, fc3_b=fc3_b,
        conv1_w=c1w, conv1_b=c1b, conv2_w=c2w, conv2_b=c2b,
        batch_num=10,
    )
    out = kernel(**ins)
    print("kernel out", out.shape, out.dtype, float(np.abs(out).max()))
